# revision 12
# baseline (speedup 1.0000x reference)
"""Diagonal-covariance MVN negative log-likelihood loss on 8 TRN2 NeuronCores.

loss = -(1/B) * sum_b log_prob_b
     = 0.5 * ( sum_{b,d} [ (t-mu)^2/sigma + ln(sigma) ] / B  +  D*ln(2pi) )

Sharding: pure data parallel over the batch dim (B=16384 -> 2048 rows/core).
Each core streams its 3x16MB shard through a raw-Bass 3-engine pipeline
(SP issues DMA loads, ACT does ln/exp/square with free row-sum accumulation,
DVE does subtract/multiply), and outputs a tiny (128, 32) stats tile of
per-partition partial sums. The final scalar reduction happens on the host
in float64.

Raw Bass (not Tile) because this toolchain's walrus rejects instructions
carrying more than one attached sync wait; manual standalone wait_ge
instructions sidestep that.
"""

import sys
from contextlib import ExitStack

for _p in ("/opt/trn_rl_repo", "/opt/pypackages"):
    if _p not in sys.path:
        sys.path.insert(0, _p)

import numpy as np

import concourse.bass as bass
import concourse.mybir as mybir
from concourse.bass_utils import run_bass_kernel_spmd

B, D = 16384, 2048
N_CORES = 8
RPC = B // N_CORES          # rows per core = 2048
P = 128                     # SBUF partitions
NT = RPC // P               # 16 row-tiles per core
NB = 2                      # buffers per stream (double buffering)
LOG_2PI = float(np.log(2.0 * np.pi))

TRACE = False
LAST_RESULTS = None

_nc_cache = None


def build_nc(repeats: int = 1) -> bass.Bass:
    """repeats>1 re-runs the identical body R times (idempotent: activation
    accum_out overwrites) — used only by the benchmark's differential timing."""
    nc = bass.Bass()
    f32 = mybir.dt.float32
    F = mybir.ActivationFunctionType
    mu = nc.dram_tensor("mu", [RPC, D], f32, kind="ExternalInput")
    sg = nc.dram_tensor("sigma", [RPC, D], f32, kind="ExternalInput")
    tg = nc.dram_tensor("target", [RPC, D], f32, kind="ExternalInput")
    # stats[:, 0:NT]   = per-partition sums of ln(sigma) for tile i
    # stats[:, NT:2NT] = per-partition sums of (t-mu)^2/sigma for tile i
    stats = nc.dram_tensor("stats", [P, 2 * NT], f32, kind="ExternalOutput")

    mu3 = mu[:, :].rearrange("(n p) d -> n p d", p=P)
    sg3 = sg[:, :].rearrange("(n p) d -> n p d", p=P)
    tg3 = tg[:, :].rearrange("(n p) d -> n p d", p=P)

    with ExitStack() as ctx:
        def bufs(name):
            return [
                ctx.enter_context(nc.sbuf_tensor(f"{name}{j}", [P, D], f32))
                for j in range(NB)
            ]

        sgt, mut, tgt = bufs("sgt"), bufs("mut"), bufs("tgt")
        lt, rst, dft, wt, qt = bufs("lt"), bufs("rst"), bufs("dft"), bufs("wt"), bufs("qt")
        stats_t = ctx.enter_context(nc.sbuf_tensor("stats_t", [P, 2 * NT], f32))

        # One sem per (stream, buffer slot): at most one in-flight increment
        # each, so waits always target the sem's final value (HWDGE
        # completions across dma_starts are not ordered).
        sg_sem = [ctx.enter_context(nc.semaphore(f"sg_sem{j}")) for j in range(NB)]
        mu_sem = [ctx.enter_context(nc.semaphore(f"mu_sem{j}")) for j in range(NB)]
        tg_sem = [ctx.enter_context(nc.semaphore(f"tg_sem{j}")) for j in range(NB)]
        asem = ctx.enter_context(nc.semaphore("asem"))   # +1 per ACT op (3/iter)
        vsem = ctx.enter_context(nc.semaphore("vsem"))   # +1 per DVE op (2/iter)
        ssem = ctx.enter_context(nc.semaphore("ssem"))   # +16 final store
        block = ctx.enter_context(nc.Block())

        NK = repeats * NT

        @block.sync
        def _(sync):
            for k in range(NK):
                i, p = k % NT, k % NB
                if k >= NB:
                    # buffer recycle: iter k-NB consumers must be done
                    sync.wait_ge(asem, 3 * (k - NB) + 1)   # Ln_{k-NB} read sgt[p]
                    sync.wait_ge(vsem, 2 * (k - NB) + 1)   # sub_{k-NB} read mut/tgt[p]
                sync.dma_start(out=sgt[p][:, :], in_=sg3[i, :, :]).then_inc(sg_sem[p], 16)
                sync.dma_start(out=mut[p][:, :], in_=mu3[i, :, :]).then_inc(mu_sem[p], 16)
                sync.dma_start(out=tgt[p][:, :], in_=tg3[i, :, :]).then_inc(tg_sem[p], 16)
            sync.wait_ge(asem, 3 * NK)                     # all ACT done
            sync.dma_start(out=stats[:, :], in_=stats_t[:, :]).then_inc(ssem, 16)
            sync.wait_ge(ssem, 16)

        @block.scalar
        def _(scalar):
            for k in range(NK):
                i, p = k % NT, k % NB
                scalar.wait_ge(sg_sem[p], 16 * (k // NB + 1))  # sigma_k loaded
                nc.scalar.activation(
                    lt[p][:, :], sgt[p][:, :], F.Ln,
                    accum_out=stats_t[:, i : i + 1],
                ).then_inc(asem, 1)                        # tick 3k+1
                if k >= NB:
                    scalar.wait_ge(vsem, 2 * (k - NB) + 2)  # mul_{k-NB} read rst[p]
                scalar.wait_ge(asem, 3 * k + 1)            # Ln_k wrote lt[p] (same-engine RAW)
                nc.scalar.activation(
                    rst[p][:, :], lt[p][:, :], F.Exp, scale=-0.5,
                ).then_inc(asem, 1)                        # tick 3k+2
                scalar.wait_ge(vsem, 2 * k + 2)            # mul_k wrote wt[p]
                nc.scalar.activation(
                    qt[p][:, :], wt[p][:, :], F.Square,
                    accum_out=stats_t[:, NT + i : NT + i + 1],
                ).then_inc(asem, 1)                        # tick 3k+3

        @block.vector
        def _(vector):
            for k in range(NK):
                p = k % NB
                vector.wait_ge(mu_sem[p], 16 * (k // NB + 1))  # mu_k loaded
                vector.wait_ge(tg_sem[p], 16 * (k // NB + 1))  # tg_k loaded
                nc.vector.tensor_sub(
                    dft[p][:, :], tgt[p][:, :], mut[p][:, :]
                ).then_inc(vsem, 1)                        # tick 2k+1
                vector.wait_ge(asem, 3 * k + 2)            # Exp_k wrote rst[p]
                vector.wait_ge(vsem, 2 * k + 1)            # sub_k wrote dft[p] (same-engine RAW)
                nc.vector.tensor_mul(
                    wt[p][:, :], dft[p][:, :], rst[p][:, :]
                ).then_inc(vsem, 1)                        # tick 2k+2

    return nc


def build_nc_v2(repeats: int = 1, nb: int = 3) -> bass.Bass:
    """v2: 4 SBUF tile groups with buffer reuse, nb-deep pipelining, sigma
    loads issued from the ACT engine's own HWDGE ring (mu/target on SP's),
    in-place DVE ops.

    Per iteration k (slot p = k % nb), tile index i = k % NT:
      SP :  load mut[p] <- mu_i, tgt[p] <- tg_i        (after Square_{k-nb})
      ACT:  Ln:  lt[p] <- ln(sgt[p])          accum -> stats[:, i]
            Exp: sgt[p] <- exp(-0.5*lt[p])    (rs overwrites sigma)
            Square: mut[p] <- (tgt[p])^2      accum -> stats[:, NT+i]
            issue load sgt[p] <- sg_{k+nb}    (rs dead after mul_k)
      DVE:  sub: tgt[p] <- tgt[p] - mut[p]    (diff, in place)
            mul: tgt[p] <- tgt[p] * sgt[p]    (w = diff * rs, in place)
    """
    nc = bass.Bass()
    f32 = mybir.dt.float32
    F = mybir.ActivationFunctionType
    mu = nc.dram_tensor("mu", [RPC, D], f32, kind="ExternalInput")
    sg = nc.dram_tensor("sigma", [RPC, D], f32, kind="ExternalInput")
    tg = nc.dram_tensor("target", [RPC, D], f32, kind="ExternalInput")
    stats = nc.dram_tensor("stats", [P, 2 * NT], f32, kind="ExternalOutput")

    mu3 = mu[:, :].rearrange("(n p) d -> n p d", p=P)
    sg3 = sg[:, :].rearrange("(n p) d -> n p d", p=P)
    tg3 = tg[:, :].rearrange("(n p) d -> n p d", p=P)

    NK = repeats * NT

    with ExitStack() as ctx:
        def bufs(name):
            return [
                ctx.enter_context(nc.sbuf_tensor(f"{name}{j}", [P, D], f32))
                for j in range(nb)
            ]

        sgt, mut, tgt, lt = bufs("sgt"), bufs("mut"), bufs("tgt"), bufs("lt")
        stats_t = ctx.enter_context(nc.sbuf_tensor("stats_t", [P, 2 * NT], f32))

        sg_sem = [ctx.enter_context(nc.semaphore(f"sg_sem{j}")) for j in range(nb)]
        mu_sem = [ctx.enter_context(nc.semaphore(f"mu_sem{j}")) for j in range(nb)]
        tg_sem = [ctx.enter_context(nc.semaphore(f"tg_sem{j}")) for j in range(nb)]
        asem = ctx.enter_context(nc.semaphore("asem"))   # +1 per activation
        vsem = ctx.enter_context(nc.semaphore("vsem"))   # +1 per DVE op
        ssem = ctx.enter_context(nc.semaphore("ssem"))   # +16 final store
        block = ctx.enter_context(nc.Block())

        @block.sync
        def _(sync):
            for k in range(NK):
                i, p = k % NT, k % nb
                if k >= nb:
                    # Square_{k-nb} read tgt[p] (w) and wrote mut[p] (q):
                    # both buffers free once it completes.
                    sync.wait_ge(asem, 3 * (k - nb) + 3)
                sync.dma_start(out=mut[p][:, :], in_=mu3[i, :, :]).then_inc(mu_sem[p], 16)
                sync.dma_start(out=tgt[p][:, :], in_=tg3[i, :, :]).then_inc(tg_sem[p], 16)
            sync.wait_ge(asem, 3 * NK)
            sync.dma_start(out=stats[:, :], in_=stats_t[:, :]).then_inc(ssem, 16)
            sync.wait_ge(ssem, 16)

        @block.scalar
        def _(scalar):
            # prologue: first nb sigma loads on the ACT ring
            for j in range(min(nb, NK)):
                nc.scalar.dma_start(
                    out=sgt[j][:, :], in_=sg3[j % NT, :, :]
                ).then_inc(sg_sem[j], 16)
            for k in range(NK):
                i, p = k % NT, k % nb
                scalar.wait_ge(sg_sem[p], 16 * (k // nb + 1))   # sigma_k landed
                nc.scalar.activation(
                    lt[p][:, :], sgt[p][:, :], F.Ln,
                    accum_out=stats_t[:, i : i + 1],
                ).then_inc(asem, 1)                             # tick 3k+1
                scalar.wait_ge(asem, 3 * k + 1)                 # Ln_k done (RAW lt)
                nc.scalar.activation(
                    sgt[p][:, :], lt[p][:, :], F.Exp, scale=-0.5,
                ).then_inc(asem, 1)                             # tick 3k+2
                scalar.wait_ge(vsem, 2 * k + 2)                 # mul_k done (w ready, rs dead)
                nc.scalar.activation(
                    mut[p][:, :], tgt[p][:, :], F.Square,
                    accum_out=stats_t[:, NT + i : NT + i + 1],
                ).then_inc(asem, 1)                             # tick 3k+3
                if k + nb < NK:
                    nc.scalar.dma_start(
                        out=sgt[p][:, :], in_=sg3[(k + nb) % NT, :, :]
                    ).then_inc(sg_sem[p], 16)

        @block.vector
        def _(vector):
            for k in range(NK):
                p = k % nb
                vector.wait_ge(mu_sem[p], 16 * (k // nb + 1))
                vector.wait_ge(tg_sem[p], 16 * (k // nb + 1))
                nc.vector.tensor_sub(
                    tgt[p][:, :], tgt[p][:, :], mut[p][:, :]
                ).then_inc(vsem, 1)                             # tick 2k+1
                vector.wait_ge(asem, 3 * k + 2)                 # Exp_k done (rs ready)
                vector.wait_ge(vsem, 2 * k + 1)                 # sub_k done (RAW)
                nc.vector.tensor_mul(
                    tgt[p][:, :], tgt[p][:, :], sgt[p][:, :]
                ).then_inc(vsem, 1)                             # tick 2k+2

    return nc


BUILDER = build_nc_v2


def _get_nc() -> bass.Bass:
    global _nc_cache
    if _nc_cache is None:
        _nc_cache = BUILDER()
    return _nc_cache


def kernel(mu: np.ndarray, sigma: np.ndarray, target: np.ndarray) -> np.ndarray:
    global LAST_RESULTS
    mu = np.ascontiguousarray(np.asarray(mu, dtype=np.float32))
    sigma = np.ascontiguousarray(np.asarray(sigma, dtype=np.float32))
    target = np.ascontiguousarray(np.asarray(target, dtype=np.float32))
    assert mu.shape == (B, D) and sigma.shape == (B, D) and target.shape == (B, D)

    in_maps = []
    for c in range(N_CORES):
        s = slice(c * RPC, (c + 1) * RPC)
        in_maps.append({"mu": mu[s], "sigma": sigma[s], "target": target[s]})

    nc = _get_nc()
    res = run_bass_kernel_spmd(nc, in_maps, list(range(N_CORES)), trace=TRACE)
    LAST_RESULTS = res

    total = 0.0
    for r in res.results:
        total += float(r["stats"].astype(np.float64).sum())
    loss = 0.5 * (total / B + D * LOG_2PI)
    return np.asarray(loss, dtype=np.float32)
